# revision 8
# baseline (speedup 1.0000x reference)
"""CARAFE content-aware upsampling (scale=2, K=5, encoder 3x3) on 8 TRN2 NeuronCores.

Sharding: 8 shards = batch(4) x H-halves(2), pure data parallel (1-row x halo
per shard handled host-side). Channel-major fp16 pipeline:

  1. compress 1x1 conv      : PE matmul (fp16 in, fp32 PSUM acc)
  2. encoder 3x3 conv       : 9 accumulating PE matmuls on a zero-padded grid
  3. e = exp(enc + b)       : ACT, fp16
  4. combined masks Mu      : pixel-shuffle + softmax-regroup collapse into one
                              small PE matmul  Mu[40,pix] = A^T @ e
                              (36 shifted-tap masses + 4 softmax denominators)
  5. r = exp(-ln S)         : ACT (softmax normalizer, deferred to the end)
  6. mask broadcast         : Mu bounced to DRAM, then one DMA per subgrid
                              broadcast-loads [128, 10, pix] fp16 (taps + r)
  7. reassembly             : 9 contiguous fp16 DVE multiplies (2x mode) per
                              subgrid; 9-term accumulation on PE via stationary
                              identity matmuls into PSUM (fp32)
  8. out = acc * r          : DVE, written subgrid-strided; SWDGE DMA casts
                              fp16 -> fp32 on store
"""

import numpy as np

SCALE, KK, EK = 2, 5, 3
B, C, H, W = 4, 128, 64, 64
CC, KC = 64, 100
HS = H // 2          # 32 interior rows per shard
PIX = HS * W
NCORES = 8
TAPS = [(dy, dx) for dy in (-1, 0, 1) for dx in (-1, 0, 1)]

_PROGRAM = None


def _build_A():
    A = np.zeros((KC, 40), dtype=np.float32)
    for r1 in range(2):
        for r2 in range(2):
            q = 2 * r1 + r2
            for i in range(KK):
                for j in range(KK):
                    dy = (r1 + i - 2) // 2
                    dx = (r2 + j - 2) // 2
                    tidx = (dy + 1) * 3 + (dx + 1)
                    A[4 * (5 * i + j) + q, q * 9 + tidx] += 1.0
            A[np.arange(q, KC, 4), 36 + q] = 1.0
    return A


def _build_program():
    import concourse.bass as bass
    import concourse.tile as tile
    from concourse.tile import add_dep_helper
    from concourse import bacc, mybir

    f32 = mybir.dt.float32
    f16 = mybir.dt.float16
    AF = mybir.ActivationFunctionType

    nc = bacc.Bacc("TRN2", target_bir_lowering=False, debug=False,
                   num_devices=NCORES)

    xin = nc.declare_dram_parameter("xs", [C, HS + 2, W], f32, isOutput=False)
    cwT = nc.declare_dram_parameter("comp_wT", [C, CC], f16, isOutput=False)
    cb = nc.declare_dram_parameter("comp_b", [CC, 1], f32, isOutput=False)
    ewT = nc.declare_dram_parameter("enc_wT", [CC, 9, KC], f16, isOutput=False)
    eb = nc.declare_dram_parameter("enc_b", [KC, 1], f32, isOutput=False)
    out = nc.declare_dram_parameter("out", [C, 2 * HS, 2 * W], f32, isOutput=True)

    A_dram = nc.inline_tensor(_build_A().astype(np.float16), name="A_cmb")
    I_dram = nc.inline_tensor(np.eye(128, dtype=np.float16), name="ident")

    mu_dram = nc.dram_tensor("mu_bounce", [4, 10, HS, W], f16)

    with tile.TileContext(nc) as tc:
        with (
            tc.tile_pool(name="singles", bufs=1) as singles,
            tc.tile_pool(name="work", bufs=4) as work,
            tc.tile_pool(name="mc", bufs=2) as mc,
        ):
            # persistent SBUF
            x16 = [singles.tile([C, HS + 2, W], f16, tag=f"x16_{d}",
                                name=f"x16_{d}")
                   for d in range(3)]  # dx = -1, 0, +1 pre-shifted copies
            k1_pad = singles.tile([CC, HS + 2, W + 2], f16, tag="k1_pad")
            e_sb = singles.tile([KC, HS, W], f16, tag="e_sb")
            mu16 = singles.tile([36, HS, W], f16, tag="mu16")
            r16 = singles.tile([4, HS, W], f16, tag="r16")
            lnS = singles.tile([4, HS, W], f32, tag="lnS")
            out32 = singles.tile([C, HS, 2, W, 2], f32, tag="out32")
            cwT_sb = singles.tile([C, CC], f16, tag="cwT")
            cb_sb = singles.tile([CC, 1], f32, tag="cb")
            ewT_sb = singles.tile([CC, 9, KC], f16, tag="ewT")
            eb_sb = singles.tile([KC, 1], f32, tag="eb")
            A_sb = singles.tile([KC, 40], f16, tag="A_sb")
            id_sb = singles.tile([128, 128], f16, tag="id_sb")

            nc.vector.memset(x16[0][:, :, 0:1], 0.0)
            nc.vector.memset(x16[2][:, :, W - 1 : W], 0.0)
            nc.vector.memset(k1_pad[:, :, 0:1], 0.0)
            nc.vector.memset(k1_pad[:, :, W + 1 : W + 2], 0.0)

            # x load with fp32 -> fp16 cast (SWDGE); build dx-shifted copies on ACT
            nc.gpsimd.dma_start(out=x16[1], in_=xin[:])
            nc.sync.dma_start(out=cwT_sb, in_=cwT[:])
            nc.sync.dma_start(out=cb_sb, in_=cb[:])
            nc.sync.dma_start(out=ewT_sb, in_=ewT[:])
            nc.sync.dma_start(out=eb_sb, in_=eb[:])
            nc.sync.dma_start(out=A_sb, in_=A_dram[:])
            nc.sync.dma_start(out=id_sb, in_=I_dram[:])

            nc.scalar.copy(x16[0][:, :, 1:W], x16[1][:, :, 0 : W - 1])
            nc.scalar.copy(x16[2][:, :, 0 : W - 1], x16[1][:, :, 1:W])

            with tc.tile_pool(name="ps_a", bufs=2, space="PSUM") as ps_a:
                # stage 1: compress conv over all 34 rows
                row_chunks = [(0, 8), (8, 16), (16, 24), (24, 32), (32, 34)]
                for r0, r1_ in row_chunks:
                    ps = ps_a.tile([CC, r1_ - r0, W], f32, tag="ps")
                    nc.tensor.matmul(ps, cwT_sb, x16[1][:, r0:r1_, :],
                                     start=True, stop=True)
                    nc.scalar.add(k1_pad[:, r0:r1_, 1 : 1 + W], ps, cb_sb)

                # stage 2+3: encoder conv + exp
                for cchunk in range(4):
                    y0 = 8 * cchunk
                    ps = ps_a.tile([KC, 8, W], f32, tag="ps")
                    for di in range(3):
                        for dj in range(3):
                            tap = di * 3 + dj
                            nc.tensor.matmul(
                                ps, ewT_sb[:, tap, :],
                                k1_pad[:, y0 + di : y0 + di + 8, dj : dj + W],
                                start=(tap == 0), stop=(tap == 8))
                    nc.scalar.activation(e_sb[:, y0 : y0 + 8, :], ps, AF.Exp,
                                         bias=eb_sb, scale=1.0)

                # stage 4: combined masses + softmax denominators
                for cchunk in range(4):
                    y0 = 8 * cchunk
                    ps = ps_a.tile([36, 8, W], f32, tag="ps")
                    nc.tensor.matmul(ps, A_sb[:, 0:36], e_sb[:, y0 : y0 + 8, :],
                                     start=True, stop=True)
                    nc.scalar.copy(mu16[:, y0 : y0 + 8, :], ps)
                    ps_s = ps_a.tile([4, 8, W], f32, tag="ps_s")
                    nc.tensor.matmul(ps_s, A_sb[:, 36:40], e_sb[:, y0 : y0 + 8, :],
                                     start=True, stop=True)
                    nc.scalar.activation(lnS[:, y0 : y0 + 8, :], ps_s, AF.Ln)
                nc.scalar.activation(r16, lnS, AF.Exp, scale=-1.0)

            # stage 6 prep: bounce masks to DRAM (chunked, pipelined)
            bounce = []
            for cchunk in range(4):
                y0 = 8 * cchunk
                dst_m = bass.AP(tensor=mu_dram, offset=y0 * W,
                                ap=[[10 * PIX, 4], [PIX, 9], [W, 8], [1, W]])
                dst_r = bass.AP(tensor=mu_dram, offset=9 * PIX + y0 * W,
                                ap=[[10 * PIX, 4], [W, 8], [1, W]])
                bm = nc.sync.dma_start(out=dst_m, in_=mu16[:, y0 : y0 + 8, :])
                br = nc.sync.dma_start(out=dst_r, in_=r16[:, y0 : y0 + 8, :])
                bounce.append((bm, br))

            with tc.tile_pool(name="ps_b", bufs=2, space="PSUM") as ps_b:
                for r1 in range(2):
                    for r2 in range(2):
                        q = 2 * r1 + r2
                        mcast = mc.tile([128, 10, HS, W], f16, tag="mcast")
                        for cchunk in range(4):
                            y0 = 8 * cchunk
                            src = bass.AP(
                                tensor=mu_dram, offset=q * 10 * PIX + y0 * W,
                                ap=[[0, 128], [PIX, 10], [W, 8], [1, W]])
                            bc = nc.sync.dma_start(
                                out=mcast[:, :, y0 : y0 + 8, :], in_=src)
                            bm, br = bounce[cchunk]
                            add_dep_helper(bc.ins, bm.ins, sync=True,
                                           reason="bcast after bounce chunk")
                            add_dep_helper(bc.ins, br.ins, sync=True,
                                           reason="bcast after bounce chunk")

                        acc = ps_b.tile([C, HS, W], f32, tag="acc")
                        for tidx, (dy, dx) in enumerate(TAPS):
                            xw = x16[dx + 1][:, 1 + dy : 1 + dy + HS, :]
                            tmp = work.tile([C, HS, W], f16, tag="tmp")
                            nc.vector.tensor_mul(tmp, xw, mcast[:, tidx])
                            for cchunk in range(4):
                                y0 = 8 * cchunk
                                nc.tensor.matmul(
                                    acc[:, y0 : y0 + 8, :], id_sb,
                                    tmp[:, y0 : y0 + 8, :],
                                    start=(tidx == 0), stop=(tidx == 8),
                                    skip_group_check=True)
                        nc.vector.tensor_mul(out32[:, :, r1, :, r2], acc,
                                             mcast[:, 9])

            nc.sync.dma_start(out=out[:], in_=out32)

    nc.compile()
    return nc


def _get_program():
    global _PROGRAM
    if _PROGRAM is None:
        _PROGRAM = _build_program()
    return _PROGRAM


def _shard_inputs(x, comp_w, comp_b, enc_w, enc_b):
    comp_wT = np.ascontiguousarray(comp_w[:, :, 0, 0].T.astype(np.float16))
    enc_wT = np.ascontiguousarray(
        np.transpose(enc_w.reshape(KC, CC, 9), (1, 2, 0)).astype(np.float16))
    cb = np.ascontiguousarray(comp_b.astype(np.float32).reshape(CC, 1))
    eb = np.ascontiguousarray(enc_b.astype(np.float32).reshape(KC, 1))
    in_maps = []
    for core in range(NCORES):
        b, h = divmod(core, 2)
        xs = np.zeros((C, HS + 2, W), dtype=np.float32)
        lo = h * HS - 1
        s0, s1 = max(0, lo), min(H, lo + HS + 2)
        xs[:, s0 - lo : s1 - lo, :] = x[b, :, s0:s1, :]
        in_maps.append({
            "xs": np.ascontiguousarray(xs),
            "comp_wT": comp_wT,
            "comp_b": cb,
            "enc_wT": enc_wT,
            "enc_b": eb,
        })
    return in_maps


def _run(inputs, trace=False):
    from concourse.bass_utils import run_bass_kernel_spmd

    nc = _get_program()
    in_maps = _shard_inputs(**inputs)
    res = run_bass_kernel_spmd(nc, in_maps, list(range(NCORES)), trace=trace)
    out = np.empty((B, C, 2 * H, 2 * W), dtype=np.float32)
    for core in range(NCORES):
        b, h = divmod(core, 2)
        out[b, :, h * 2 * HS : (h + 1) * 2 * HS, :] = res.results[core]["out"]
    return out, res.exec_time_ns


def kernel(x, comp_w, comp_b, enc_w, enc_b):
    out, _ = _run(dict(x=np.asarray(x), comp_w=np.asarray(comp_w),
                       comp_b=np.asarray(comp_b), enc_w=np.asarray(enc_w),
                       enc_b=np.asarray(enc_b)))
    return out


# revision 9
# speedup vs baseline: 1.0256x; 1.0256x over previous
"""CARAFE content-aware upsampling (scale=2, K=5, encoder 3x3) on 8 TRN2 NeuronCores.

Sharding: 8 shards = batch(4) x H-halves(2), pure data parallel (1-row x halo
per shard handled host-side). Channel-major fp16 pipeline:

  1. compress 1x1 conv      : PE matmul (fp16 in, fp32 PSUM acc)
  2. encoder 3x3 conv       : 9 accumulating PE matmuls on a zero-padded grid
  3. e = exp(enc + b)       : ACT, fp16
  4. combined masks Mu      : pixel-shuffle + softmax-regroup collapse into one
                              small PE matmul  Mu[40,pix] = A^T @ e
                              (36 shifted-tap masses + 4 softmax denominators)
  5. r = exp(-ln S)         : ACT (softmax normalizer, deferred to the end)
  6. mask broadcast         : Mu bounced to DRAM, then one DMA per subgrid
                              broadcast-loads [128, 10, pix] fp16 (taps + r)
  7. reassembly             : 9 contiguous fp16 DVE multiplies (2x mode) per
                              subgrid; 9-term accumulation on PE via stationary
                              identity matmuls into PSUM (fp32)
  8. out = acc * r          : DVE, written subgrid-strided; SWDGE DMA casts
                              fp16 -> fp32 on store
"""

import numpy as np

SCALE, KK, EK = 2, 5, 3
B, C, H, W = 4, 128, 64, 64
CC, KC = 64, 100
HS = H // 2          # 32 interior rows per shard
PIX = HS * W
NCORES = 8
TAPS = [(dy, dx) for dy in (-1, 0, 1) for dx in (-1, 0, 1)]

_PROGRAM = None


def _build_A():
    A = np.zeros((KC, 40), dtype=np.float32)
    for r1 in range(2):
        for r2 in range(2):
            q = 2 * r1 + r2
            for i in range(KK):
                for j in range(KK):
                    dy = (r1 + i - 2) // 2
                    dx = (r2 + j - 2) // 2
                    tidx = (dy + 1) * 3 + (dx + 1)
                    A[4 * (5 * i + j) + q, q * 9 + tidx] += 1.0
            A[np.arange(q, KC, 4), 36 + q] = 1.0
    return A


def _build_program():
    import concourse.bass as bass
    import concourse.tile as tile
    from concourse.tile import add_dep_helper
    from concourse import bacc, mybir

    f32 = mybir.dt.float32
    f16 = mybir.dt.float16
    AF = mybir.ActivationFunctionType

    nc = bacc.Bacc("TRN2", target_bir_lowering=False, debug=False,
                   num_devices=NCORES)

    xin = nc.declare_dram_parameter("xs", [C, HS + 2, W], f32, isOutput=False)
    cwT = nc.declare_dram_parameter("comp_wT", [C, CC], f16, isOutput=False)
    cb = nc.declare_dram_parameter("comp_b", [CC, 1], f32, isOutput=False)
    ewT = nc.declare_dram_parameter("enc_wT", [CC, 9, KC], f16, isOutput=False)
    eb = nc.declare_dram_parameter("enc_b", [KC, 1], f32, isOutput=False)
    out = nc.declare_dram_parameter("out", [C, 2 * HS, 2 * W], f32, isOutput=True)

    A_dram = nc.inline_tensor(_build_A().astype(np.float16), name="A_cmb")
    I_dram = nc.inline_tensor(np.eye(128, dtype=np.float16), name="ident")

    mu_dram = nc.dram_tensor("mu_bounce", [4, 10, HS, W], f16)

    with tile.TileContext(nc) as tc:
        with (
            tc.tile_pool(name="singles", bufs=1) as singles,
            tc.tile_pool(name="work", bufs=4) as work,
            tc.tile_pool(name="mc", bufs=2) as mc,
        ):
            # persistent SBUF
            x16 = [singles.tile([C, HS + 2, W], f16, tag=f"x16_{d}",
                                name=f"x16_{d}")
                   for d in range(3)]  # dx = -1, 0, +1 pre-shifted copies
            k1_pad = singles.tile([CC, HS + 2, W + 2], f16, tag="k1_pad")
            e_sb = singles.tile([KC, HS, W], f16, tag="e_sb")
            mu16 = singles.tile([36, HS, W], f16, tag="mu16")
            r16 = singles.tile([4, HS, W], f16, tag="r16")
            lnS = singles.tile([4, HS, W], f32, tag="lnS")
            out32 = singles.tile([C, HS, 2, W, 2], f32, tag="out32")
            cwT_sb = singles.tile([C, CC], f16, tag="cwT")
            cb_sb = singles.tile([CC, 1], f32, tag="cb")
            ewT_sb = singles.tile([CC, 9, KC], f16, tag="ewT")
            eb_sb = singles.tile([KC, 1], f32, tag="eb")
            A_sb = singles.tile([KC, 40], f16, tag="A_sb")
            id_sb = singles.tile([128, 128], f16, tag="id_sb")

            nc.vector.memset(x16[0][:, :, 0:1], 0.0)
            nc.vector.memset(x16[2][:, :, W - 1 : W], 0.0)
            nc.vector.memset(k1_pad[:, :, 0:1], 0.0)
            nc.vector.memset(k1_pad[:, :, W + 1 : W + 2], 0.0)

            # x load with fp32 -> fp16 cast (SWDGE); build dx-shifted copies on ACT
            nc.gpsimd.dma_start(out=x16[1], in_=xin[:])
            nc.sync.dma_start(out=cwT_sb, in_=cwT[:])
            nc.sync.dma_start(out=cb_sb, in_=cb[:])
            nc.sync.dma_start(out=ewT_sb, in_=ewT[:])
            nc.sync.dma_start(out=eb_sb, in_=eb[:])
            nc.sync.dma_start(out=A_sb, in_=A_dram[:])
            nc.sync.dma_start(out=id_sb, in_=I_dram[:])

            nc.scalar.copy(x16[0][:, :, 1:W], x16[1][:, :, 0 : W - 1])
            nc.scalar.copy(x16[2][:, :, 0 : W - 1], x16[1][:, :, 1:W])

            with tc.tile_pool(name="ps_a", bufs=2, space="PSUM") as ps_a:
                # stage 1: compress conv over all 34 rows
                row_chunks = [(0, 8), (8, 16), (16, 24), (24, 32), (32, 34)]
                for r0, r1_ in row_chunks:
                    ps = ps_a.tile([CC, r1_ - r0, W], f32, tag="ps")
                    nc.tensor.matmul(ps, cwT_sb, x16[1][:, r0:r1_, :],
                                     start=True, stop=True)
                    nc.scalar.add(k1_pad[:, r0:r1_, 1 : 1 + W], ps, cb_sb)

                # stage 2+3: encoder conv + exp
                for cchunk in range(4):
                    y0 = 8 * cchunk
                    ps = ps_a.tile([KC, 8, W], f32, tag="ps")
                    for di in range(3):
                        for dj in range(3):
                            tap = di * 3 + dj
                            nc.tensor.matmul(
                                ps, ewT_sb[:, tap, :],
                                k1_pad[:, y0 + di : y0 + di + 8, dj : dj + W],
                                start=(tap == 0), stop=(tap == 8))
                    nc.scalar.activation(e_sb[:, y0 : y0 + 8, :], ps, AF.Exp,
                                         bias=eb_sb, scale=1.0)

                # stage 4: combined masses + softmax denominators
                for cchunk in range(4):
                    y0 = 8 * cchunk
                    ps = ps_a.tile([36, 8, W], f32, tag="ps")
                    nc.tensor.matmul(ps, A_sb[:, 0:36], e_sb[:, y0 : y0 + 8, :],
                                     start=True, stop=True)
                    nc.scalar.copy(mu16[:, y0 : y0 + 8, :], ps)
                    ps_s = ps_a.tile([4, 8, W], f32, tag="ps_s")
                    nc.tensor.matmul(ps_s, A_sb[:, 36:40], e_sb[:, y0 : y0 + 8, :],
                                     start=True, stop=True)
                    nc.scalar.activation(lnS[:, y0 : y0 + 8, :], ps_s, AF.Ln)
                nc.scalar.activation(r16, lnS, AF.Exp, scale=-1.0)

            # stage 6 prep: bounce masks to DRAM (chunked, pipelined)
            bounce = []
            for cchunk in range(4):
                y0 = 8 * cchunk
                dst_m = bass.AP(tensor=mu_dram, offset=y0 * W,
                                ap=[[10 * PIX, 4], [PIX, 9], [W, 8], [1, W]])
                dst_r = bass.AP(tensor=mu_dram, offset=9 * PIX + y0 * W,
                                ap=[[10 * PIX, 4], [W, 8], [1, W]])
                bm = nc.sync.dma_start(out=dst_m, in_=mu16[:, y0 : y0 + 8, :])
                br = nc.sync.dma_start(out=dst_r, in_=r16[:, y0 : y0 + 8, :])
                bounce.append((bm, br))

            with tc.tile_pool(name="ps_b", bufs=2, space="PSUM") as ps_b:
                for r1 in range(2):
                    for r2 in range(2):
                        q = 2 * r1 + r2
                        mcast = mc.tile([128, 10, HS, W], f16, tag="mcast")
                        for cchunk in range(4):
                            y0 = 8 * cchunk
                            src = bass.AP(
                                tensor=mu_dram, offset=q * 10 * PIX + y0 * W,
                                ap=[[0, 128], [PIX, 10], [W, 8], [1, W]])
                            bc = nc.gpsimd.dma_start(
                                out=mcast[:, :, y0 : y0 + 8, :], in_=src)
                            bm, br = bounce[cchunk]
                            add_dep_helper(bc.ins, bm.ins, sync=True,
                                           reason="bcast after bounce chunk")
                            add_dep_helper(bc.ins, br.ins, sync=True,
                                           reason="bcast after bounce chunk")

                        acc = ps_b.tile([C, HS, W], f32, tag="acc")
                        for tidx, (dy, dx) in enumerate(TAPS):
                            xw = x16[dx + 1][:, 1 + dy : 1 + dy + HS, :]
                            tmp = work.tile([C, HS, W], f16, tag="tmp")
                            nc.vector.tensor_mul(tmp, xw, mcast[:, tidx])
                            for cchunk in range(4):
                                y0 = 8 * cchunk
                                nc.tensor.matmul(
                                    acc[:, y0 : y0 + 8, :], id_sb,
                                    tmp[:, y0 : y0 + 8, :],
                                    start=(tidx == 0), stop=(tidx == 8),
                                    skip_group_check=True)
                        nc.vector.tensor_mul(out32[:, :, r1, :, r2], acc,
                                             mcast[:, 9])

            nc.sync.dma_start(out=out[:], in_=out32)

    nc.compile()
    return nc


def _get_program():
    global _PROGRAM
    if _PROGRAM is None:
        _PROGRAM = _build_program()
    return _PROGRAM


def _shard_inputs(x, comp_w, comp_b, enc_w, enc_b):
    comp_wT = np.ascontiguousarray(comp_w[:, :, 0, 0].T.astype(np.float16))
    enc_wT = np.ascontiguousarray(
        np.transpose(enc_w.reshape(KC, CC, 9), (1, 2, 0)).astype(np.float16))
    cb = np.ascontiguousarray(comp_b.astype(np.float32).reshape(CC, 1))
    eb = np.ascontiguousarray(enc_b.astype(np.float32).reshape(KC, 1))
    in_maps = []
    for core in range(NCORES):
        b, h = divmod(core, 2)
        xs = np.zeros((C, HS + 2, W), dtype=np.float32)
        lo = h * HS - 1
        s0, s1 = max(0, lo), min(H, lo + HS + 2)
        xs[:, s0 - lo : s1 - lo, :] = x[b, :, s0:s1, :]
        in_maps.append({
            "xs": np.ascontiguousarray(xs),
            "comp_wT": comp_wT,
            "comp_b": cb,
            "enc_wT": enc_wT,
            "enc_b": eb,
        })
    return in_maps


def _run(inputs, trace=False):
    from concourse.bass_utils import run_bass_kernel_spmd

    nc = _get_program()
    in_maps = _shard_inputs(**inputs)
    res = run_bass_kernel_spmd(nc, in_maps, list(range(NCORES)), trace=trace)
    out = np.empty((B, C, 2 * H, 2 * W), dtype=np.float32)
    for core in range(NCORES):
        b, h = divmod(core, 2)
        out[b, :, h * 2 * HS : (h + 1) * 2 * HS, :] = res.results[core]["out"]
    return out, res.exec_time_ns


def kernel(x, comp_w, comp_b, enc_w, enc_b):
    out, _ = _run(dict(x=np.asarray(x), comp_w=np.asarray(comp_w),
                       comp_b=np.asarray(comp_b), enc_w=np.asarray(enc_w),
                       enc_b=np.asarray(enc_b)))
    return out


# revision 10
# speedup vs baseline: 1.1533x; 1.1246x over previous
"""CARAFE content-aware upsampling (scale=2, K=5, encoder 3x3) on 8 TRN2 NeuronCores.

Sharding: 8 shards = batch(4) x H-halves(2), pure data parallel (1-row x halo
per shard handled host-side). Channel-major fp16 pipeline:

  1. compress 1x1 conv      : PE matmul (fp16 in, fp32 PSUM acc)
  2. encoder 3x3 conv       : 9 accumulating PE matmuls on a zero-padded grid
  3. e = exp(enc + b)       : ACT, fp16
  4. combined masks Mu      : pixel-shuffle + softmax-regroup collapse into one
                              small PE matmul  Mu[40,pix] = A^T @ e
                              (36 shifted-tap masses + 4 softmax denominators)
  5. r = exp(-ln S)         : ACT (softmax normalizer, deferred to the end)
  6. mask broadcast         : Mu bounced to DRAM, then one DMA per subgrid
                              broadcast-loads [128, 10, pix] fp16 (taps + r)
  7. reassembly             : 9 contiguous fp16 DVE multiplies (2x mode) per
                              subgrid; 9-term accumulation on PE via stationary
                              identity matmuls into PSUM (fp32)
  8. out = acc * r          : DVE, written subgrid-strided; SWDGE DMA casts
                              fp16 -> fp32 on store
"""

import numpy as np

SCALE, KK, EK = 2, 5, 3
B, C, H, W = 4, 128, 64, 64
CC, KC = 64, 100
HS = H // 2          # 32 interior rows per shard
PIX = HS * W
NCORES = 8
TAPS = [(dy, dx) for dy in (-1, 0, 1) for dx in (-1, 0, 1)]

_PROGRAM = None


def _build_A():
    A = np.zeros((KC, 40), dtype=np.float32)
    for r1 in range(2):
        for r2 in range(2):
            q = 2 * r1 + r2
            for i in range(KK):
                for j in range(KK):
                    dy = (r1 + i - 2) // 2
                    dx = (r2 + j - 2) // 2
                    tidx = (dy + 1) * 3 + (dx + 1)
                    A[4 * (5 * i + j) + q, q * 9 + tidx] += 1.0
            A[np.arange(q, KC, 4), 36 + q] = 1.0
    return A


def _build_program():
    import concourse.bass as bass
    import concourse.tile as tile
    from concourse.tile import add_dep_helper
    from concourse import bacc, mybir

    f32 = mybir.dt.float32
    f16 = mybir.dt.float16
    AF = mybir.ActivationFunctionType

    nc = bacc.Bacc("TRN2", target_bir_lowering=False, debug=False,
                   num_devices=NCORES)

    xin = nc.declare_dram_parameter("xs", [C, HS + 2, W], f32, isOutput=False)
    cwT = nc.declare_dram_parameter("comp_wT", [C, CC], f16, isOutput=False)
    cb = nc.declare_dram_parameter("comp_b", [CC, 1], f32, isOutput=False)
    ewT = nc.declare_dram_parameter("enc_wT", [CC, 9, KC], f16, isOutput=False)
    eb = nc.declare_dram_parameter("enc_b", [KC, 1], f32, isOutput=False)
    out = nc.declare_dram_parameter("out", [C, 2 * HS, 2 * W], f32, isOutput=True)

    A_dram = nc.inline_tensor(_build_A().astype(np.float16), name="A_cmb")
    I_dram = nc.inline_tensor(np.eye(128, dtype=np.float16), name="ident")

    mu_dram = nc.dram_tensor("mu_bounce", [4, 10, HS, W], f16)

    with tile.TileContext(nc) as tc:
        with (
            tc.tile_pool(name="singles", bufs=1) as singles,
            tc.tile_pool(name="work", bufs=4) as work,
            tc.tile_pool(name="mc", bufs=2) as mc,
        ):
            # persistent SBUF
            x16 = [singles.tile([C, HS + 2, W], f16, tag=f"x16_{d}",
                                name=f"x16_{d}")
                   for d in range(3)]  # dx = -1, 0, +1 pre-shifted copies
            k1_pad = singles.tile([CC, HS + 2, W + 2], f16, tag="k1_pad")
            e_sb = singles.tile([KC, HS, W], f16, tag="e_sb")
            mu16 = singles.tile([36, HS, W], f16, tag="mu16")
            r16 = singles.tile([4, HS, W], f16, tag="r16")
            lnS = singles.tile([4, HS, W], f32, tag="lnS")
            out32 = singles.tile([C, HS, 2, W, 2], f32, tag="out32")
            cwT_sb = singles.tile([C, CC], f16, tag="cwT")
            cb_sb = singles.tile([CC, 1], f32, tag="cb")
            ewT_sb = singles.tile([CC, 9, KC], f16, tag="ewT")
            eb_sb = singles.tile([KC, 1], f32, tag="eb")
            A_sb = singles.tile([KC, 40], f16, tag="A_sb")
            id_sb = singles.tile([128, 128], f16, tag="id_sb")

            nc.vector.memset(x16[0][:, :, 0:1], 0.0)
            nc.vector.memset(x16[2][:, :, W - 1 : W], 0.0)
            nc.vector.memset(k1_pad[:, :, 0:1], 0.0)
            nc.vector.memset(k1_pad[:, :, W + 1 : W + 2], 0.0)

            # x load with fp32 -> fp16 cast (SWDGE); build dx-shifted copies on ACT
            nc.gpsimd.dma_start(out=x16[1], in_=xin[:])
            nc.sync.dma_start(out=cwT_sb, in_=cwT[:])
            nc.sync.dma_start(out=cb_sb, in_=cb[:])
            nc.sync.dma_start(out=ewT_sb, in_=ewT[:])
            nc.sync.dma_start(out=eb_sb, in_=eb[:])
            nc.sync.dma_start(out=A_sb, in_=A_dram[:])
            nc.sync.dma_start(out=id_sb, in_=I_dram[:])

            nc.scalar.copy(x16[0][:, :, 1:W], x16[1][:, :, 0 : W - 1])
            nc.scalar.copy(x16[2][:, :, 0 : W - 1], x16[1][:, :, 1:W])

            with tc.tile_pool(name="ps_a", bufs=2, space="PSUM") as ps_a:
                # stage 1: compress conv over all 34 rows
                row_chunks = [(0, 8), (8, 16), (16, 24), (24, 32), (32, 34)]
                for r0, r1_ in row_chunks:
                    ps = ps_a.tile([CC, r1_ - r0, W], f32, tag="ps")
                    nc.tensor.matmul(ps, cwT_sb, x16[1][:, r0:r1_, :],
                                     start=True, stop=True)
                    nc.scalar.add(k1_pad[:, r0:r1_, 1 : 1 + W], ps, cb_sb)

                # stage 2+3: encoder conv + exp
                for cchunk in range(4):
                    y0 = 8 * cchunk
                    ps = ps_a.tile([KC, 8, W], f32, tag="ps")
                    for di in range(3):
                        for dj in range(3):
                            tap = di * 3 + dj
                            nc.tensor.matmul(
                                ps, ewT_sb[:, tap, :],
                                k1_pad[:, y0 + di : y0 + di + 8, dj : dj + W],
                                start=(tap == 0), stop=(tap == 8))
                    nc.scalar.activation(e_sb[:, y0 : y0 + 8, :], ps, AF.Exp,
                                         bias=eb_sb, scale=1.0)

                # stage 4: combined masses + softmax denominators
                for cchunk in range(4):
                    y0 = 8 * cchunk
                    ps = ps_a.tile([36, 8, W], f32, tag="ps")
                    nc.tensor.matmul(ps, A_sb[:, 0:36], e_sb[:, y0 : y0 + 8, :],
                                     start=True, stop=True)
                    nc.scalar.copy(mu16[:, y0 : y0 + 8, :], ps)
                    ps_s = ps_a.tile([4, 8, W], f32, tag="ps_s")
                    nc.tensor.matmul(ps_s, A_sb[:, 36:40], e_sb[:, y0 : y0 + 8, :],
                                     start=True, stop=True)
                    nc.scalar.activation(lnS[:, y0 : y0 + 8, :], ps_s, AF.Ln)
                nc.scalar.activation(r16, lnS, AF.Exp, scale=-1.0)

            # stage 6 prep: bounce masks to DRAM for partition-broadcast loads
            bounce_dst_m = bass.AP(tensor=mu_dram, offset=0,
                                   ap=[[10 * PIX, 4], [PIX, 9], [W, HS], [1, W]])
            bounce_dst_r = bass.AP(tensor=mu_dram, offset=9 * PIX,
                                   ap=[[10 * PIX, 4], [W, HS], [1, W]])
            bounce_m = nc.gpsimd.dma_start(out=bounce_dst_m, in_=mu16[:])
            bounce_r = nc.gpsimd.dma_start(out=bounce_dst_r, in_=r16[:])

            with tc.tile_pool(name="ps_b", bufs=2, space="PSUM") as ps_b:
                for r1 in range(2):
                    for r2 in range(2):
                        q = 2 * r1 + r2
                        mcast = mc.tile([128, 10, HS, W], f16, tag="mcast")
                        src = bass.AP(tensor=mu_dram, offset=q * 10 * PIX,
                                      ap=[[0, 128], [1, 10 * PIX]])
                        bc = nc.gpsimd.dma_start(
                            out=mcast.rearrange("p t h w -> p (t h w)"), in_=src)
                        add_dep_helper(bc.ins, bounce_m.ins, sync=True,
                                       reason="mask broadcast after dram bounce")
                        add_dep_helper(bc.ins, bounce_r.ins, sync=True,
                                       reason="r broadcast after dram bounce")

                        acc = ps_b.tile([C, HS, W], f32, tag="acc")
                        for tidx, (dy, dx) in enumerate(TAPS):
                            xw = x16[dx + 1][:, 1 + dy : 1 + dy + HS, :]
                            tmp = work.tile([C, HS, W], f16, tag="tmp")
                            nc.vector.tensor_mul(tmp, xw, mcast[:, tidx])
                            for cchunk in range(4):
                                y0 = 8 * cchunk
                                nc.tensor.matmul(
                                    acc[:, y0 : y0 + 8, :], id_sb,
                                    tmp[:, y0 : y0 + 8, :],
                                    start=(tidx == 0), stop=(tidx == 8),
                                    skip_group_check=True)
                        nc.vector.tensor_mul(out32[:, :, r1, :, r2], acc,
                                             mcast[:, 9])

            nc.sync.dma_start(out=out[:], in_=out32)

    nc.compile()
    return nc


def _get_program():
    global _PROGRAM
    if _PROGRAM is None:
        _PROGRAM = _build_program()
    return _PROGRAM


def _shard_inputs(x, comp_w, comp_b, enc_w, enc_b):
    comp_wT = np.ascontiguousarray(comp_w[:, :, 0, 0].T.astype(np.float16))
    enc_wT = np.ascontiguousarray(
        np.transpose(enc_w.reshape(KC, CC, 9), (1, 2, 0)).astype(np.float16))
    cb = np.ascontiguousarray(comp_b.astype(np.float32).reshape(CC, 1))
    eb = np.ascontiguousarray(enc_b.astype(np.float32).reshape(KC, 1))
    in_maps = []
    for core in range(NCORES):
        b, h = divmod(core, 2)
        xs = np.zeros((C, HS + 2, W), dtype=np.float32)
        lo = h * HS - 1
        s0, s1 = max(0, lo), min(H, lo + HS + 2)
        xs[:, s0 - lo : s1 - lo, :] = x[b, :, s0:s1, :]
        in_maps.append({
            "xs": np.ascontiguousarray(xs),
            "comp_wT": comp_wT,
            "comp_b": cb,
            "enc_wT": enc_wT,
            "enc_b": eb,
        })
    return in_maps


def _run(inputs, trace=False):
    from concourse.bass_utils import run_bass_kernel_spmd

    nc = _get_program()
    in_maps = _shard_inputs(**inputs)
    res = run_bass_kernel_spmd(nc, in_maps, list(range(NCORES)), trace=trace)
    out = np.empty((B, C, 2 * H, 2 * W), dtype=np.float32)
    for core in range(NCORES):
        b, h = divmod(core, 2)
        out[b, :, h * 2 * HS : (h + 1) * 2 * HS, :] = res.results[core]["out"]
    return out, res.exec_time_ns


def kernel(x, comp_w, comp_b, enc_w, enc_b):
    out, _ = _run(dict(x=np.asarray(x), comp_w=np.asarray(comp_w),
                       comp_b=np.asarray(comp_b), enc_w=np.asarray(enc_w),
                       enc_b=np.asarray(enc_b)))
    return out


# revision 11
# speedup vs baseline: 1.2285x; 1.0652x over previous
"""CARAFE content-aware upsampling (scale=2, K=5, encoder 3x3) on 8 TRN2 NeuronCores.

Sharding: 8 shards = batch(4) x H-halves(2), pure data parallel (1-row x halo
per shard handled host-side). Channel-major fp16 pipeline:

  1. compress 1x1 conv      : PE matmul (fp16 in, fp32 PSUM acc)
  2. encoder 3x3 conv       : 9 accumulating PE matmuls on a zero-padded grid
  3. e = exp(enc + b)       : ACT, fp16
  4. combined masks Mu      : pixel-shuffle + softmax-regroup collapse into one
                              small PE matmul  Mu[40,pix] = A^T @ e
                              (36 shifted-tap masses + 4 softmax denominators)
  5. r = exp(-ln S)         : ACT (softmax normalizer, deferred to the end)
  6. mask broadcast         : Mu bounced to DRAM, then one DMA per subgrid
                              broadcast-loads [128, 10, pix] fp16 (taps + r)
  7. reassembly             : 9 contiguous fp16 DVE multiplies (2x mode) per
                              subgrid; 9-term accumulation on PE via stationary
                              identity matmuls into PSUM (fp32)
  8. out = acc * r          : DVE, written subgrid-strided; SWDGE DMA casts
                              fp16 -> fp32 on store
"""

import numpy as np

SCALE, KK, EK = 2, 5, 3
B, C, H, W = 4, 128, 64, 64
CC, KC = 64, 100
HS = H // 2          # 32 interior rows per shard
PIX = HS * W
NCORES = 8
TAPS = [(dy, dx) for dy in (-1, 0, 1) for dx in (-1, 0, 1)]

_PROGRAM = None


def _build_A():
    A = np.zeros((KC, 40), dtype=np.float32)
    for r1 in range(2):
        for r2 in range(2):
            q = 2 * r1 + r2
            for i in range(KK):
                for j in range(KK):
                    dy = (r1 + i - 2) // 2
                    dx = (r2 + j - 2) // 2
                    tidx = (dy + 1) * 3 + (dx + 1)
                    A[4 * (5 * i + j) + q, q * 9 + tidx] += 1.0
            A[np.arange(q, KC, 4), 36 + q] = 1.0
    return A


def _build_program():
    import concourse.bass as bass
    import concourse.tile as tile
    from concourse.tile import add_dep_helper
    from concourse import bacc, mybir

    f32 = mybir.dt.float32
    f16 = mybir.dt.float16
    AF = mybir.ActivationFunctionType

    nc = bacc.Bacc("TRN2", target_bir_lowering=False, debug=False,
                   num_devices=NCORES)

    xin = nc.declare_dram_parameter("xs", [C, HS + 2, W], f32, isOutput=False)
    cwT = nc.declare_dram_parameter("comp_wT", [C, CC], f16, isOutput=False)
    cb = nc.declare_dram_parameter("comp_b", [CC, 1], f32, isOutput=False)
    ewT = nc.declare_dram_parameter("enc_wT", [CC, 9, KC], f16, isOutput=False)
    eb = nc.declare_dram_parameter("enc_b", [KC, 1], f32, isOutput=False)
    out = nc.declare_dram_parameter("out", [C, 2 * HS, 2 * W], f32, isOutput=True)

    A_dram = nc.inline_tensor(_build_A().astype(np.float16), name="A_cmb")
    I_dram = nc.inline_tensor(np.eye(128, dtype=np.float16), name="ident")

    mu_dram = nc.dram_tensor("mu_bounce", [4, 10, HS, W], f16)

    with tile.TileContext(nc) as tc:
        with (
            tc.tile_pool(name="singles", bufs=1) as singles,
            tc.tile_pool(name="work", bufs=4) as work,
            tc.tile_pool(name="mc", bufs=2) as mc,
        ):
            # persistent SBUF
            x16 = [singles.tile([C, HS + 2, W], f16, tag=f"x16_{d}",
                                name=f"x16_{d}")
                   for d in range(3)]  # dx = -1, 0, +1 pre-shifted copies
            k1_pad = singles.tile([CC, HS + 2, W + 2], f16, tag="k1_pad")
            e_sb = singles.tile([KC, HS, W], f16, tag="e_sb")
            mu16 = singles.tile([36, HS, W], f16, tag="mu16")
            r16 = singles.tile([4, HS, W], f16, tag="r16")
            lnS = singles.tile([4, HS, W], f32, tag="lnS")
            out32 = singles.tile([C, HS, 2, W, 2], f32, tag="out32")
            cwT_sb = singles.tile([C, CC], f16, tag="cwT")
            cb_sb = singles.tile([CC, 1], f32, tag="cb")
            ewT_sb = singles.tile([CC, 9, KC], f16, tag="ewT")
            eb_sb = singles.tile([KC, 1], f32, tag="eb")
            A_sb = singles.tile([KC, 40], f16, tag="A_sb")
            id_sb = singles.tile([128, 128], f16, tag="id_sb")

            nc.vector.memset(x16[0][:, :, 0:1], 0.0)
            nc.vector.memset(x16[2][:, :, W - 1 : W], 0.0)
            nc.vector.memset(k1_pad[:, :, 0:1], 0.0)
            nc.vector.memset(k1_pad[:, :, W + 1 : W + 2], 0.0)

            # x load with fp32 -> fp16 cast (SWDGE); build dx-shifted copies on ACT
            nc.gpsimd.dma_start(out=x16[1], in_=xin[:])
            nc.sync.dma_start(out=cwT_sb, in_=cwT[:])
            nc.sync.dma_start(out=cb_sb, in_=cb[:])
            nc.sync.dma_start(out=ewT_sb, in_=ewT[:])
            nc.sync.dma_start(out=eb_sb, in_=eb[:])
            nc.sync.dma_start(out=A_sb, in_=A_dram[:])
            nc.sync.dma_start(out=id_sb, in_=I_dram[:])

            nc.scalar.copy(x16[0][:, :, 1:W], x16[1][:, :, 0 : W - 1])
            nc.scalar.copy(x16[2][:, :, 0 : W - 1], x16[1][:, :, 1:W])

            with tc.tile_pool(name="ps_a", bufs=2, space="PSUM") as ps_a:
                # stage 1: compress conv over all 34 rows
                row_chunks = [(0, 8), (8, 16), (16, 24), (24, 32), (32, 34)]
                for r0, r1_ in row_chunks:
                    ps = ps_a.tile([CC, r1_ - r0, W], f32, tag="ps")
                    nc.tensor.matmul(ps, cwT_sb, x16[1][:, r0:r1_, :],
                                     start=True, stop=True)
                    nc.vector.tensor_scalar_add(k1_pad[:, r0:r1_, 1 : 1 + W],
                                                ps, cb_sb)

                # stage 2+3: encoder conv + exp
                for cchunk in range(4):
                    y0 = 8 * cchunk
                    ps = ps_a.tile([KC, 8, W], f32, tag="ps")
                    for di in range(3):
                        for dj in range(3):
                            tap = di * 3 + dj
                            nc.tensor.matmul(
                                ps, ewT_sb[:, tap, :],
                                k1_pad[:, y0 + di : y0 + di + 8, dj : dj + W],
                                start=(tap == 0), stop=(tap == 8))
                    nc.scalar.activation(e_sb[:, y0 : y0 + 8, :], ps, AF.Exp,
                                         bias=eb_sb, scale=1.0)

                # stage 4: combined masses + softmax denominators
                for cchunk in range(4):
                    y0 = 8 * cchunk
                    ps = ps_a.tile([36, 8, W], f32, tag="ps")
                    nc.tensor.matmul(ps, A_sb[:, 0:36], e_sb[:, y0 : y0 + 8, :],
                                     start=True, stop=True)
                    nc.vector.tensor_copy(mu16[:, y0 : y0 + 8, :], ps)
                    ps_s = ps_a.tile([4, 8, W], f32, tag="ps_s")
                    nc.tensor.matmul(ps_s, A_sb[:, 36:40], e_sb[:, y0 : y0 + 8, :],
                                     start=True, stop=True)
                    nc.scalar.activation(lnS[:, y0 : y0 + 8, :], ps_s, AF.Ln)
                    nc.scalar.activation(r16[:, y0 : y0 + 8, :],
                                         lnS[:, y0 : y0 + 8, :], AF.Exp,
                                         scale=-1.0)

            # stage 6 prep: bounce masks to DRAM for partition-broadcast loads
            bounce_dst_m = bass.AP(tensor=mu_dram, offset=0,
                                   ap=[[10 * PIX, 4], [PIX, 9], [W, HS], [1, W]])
            bounce_dst_r = bass.AP(tensor=mu_dram, offset=9 * PIX,
                                   ap=[[10 * PIX, 4], [W, HS], [1, W]])
            bounce_m = nc.gpsimd.dma_start(out=bounce_dst_m, in_=mu16[:])
            bounce_r = nc.gpsimd.dma_start(out=bounce_dst_r, in_=r16[:])

            with tc.tile_pool(name="ps_b", bufs=2, space="PSUM") as ps_b:
                for r1 in range(2):
                    for r2 in range(2):
                        q = 2 * r1 + r2
                        mcast = mc.tile([128, 10, HS, W], f16, tag="mcast")
                        mflat = mcast.rearrange("p t h w -> p (t h w)")
                        nsplit = 2 if q == 0 else 1
                        step = 10 * PIX // nsplit
                        for si in range(nsplit):
                            src = bass.AP(
                                tensor=mu_dram, offset=q * 10 * PIX + si * step,
                                ap=[[0, 128], [1, step]])
                            bc = nc.gpsimd.dma_start(
                                out=mflat[:, si * step : (si + 1) * step], in_=src)
                            add_dep_helper(bc.ins, bounce_m.ins, sync=True,
                                           reason="mask broadcast after bounce")
                            add_dep_helper(bc.ins, bounce_r.ins, sync=True,
                                           reason="r broadcast after bounce")

                        acc = ps_b.tile([C, HS, W], f32, tag="acc")
                        for tidx, (dy, dx) in enumerate(TAPS):
                            xw = x16[dx + 1][:, 1 + dy : 1 + dy + HS, :]
                            tmp = work.tile([C, HS, W], f16, tag="tmp")
                            nc.vector.tensor_mul(tmp, xw, mcast[:, tidx])
                            for cchunk in range(4):
                                y0 = 8 * cchunk
                                nc.tensor.matmul(
                                    acc[:, y0 : y0 + 8, :], id_sb,
                                    tmp[:, y0 : y0 + 8, :],
                                    start=(tidx == 0), stop=(tidx == 8),
                                    skip_group_check=True)
                        for hh in range(2):
                            hr = slice(hh * (HS // 2), (hh + 1) * (HS // 2))
                            nc.vector.tensor_mul(out32[:, hr, r1, :, r2],
                                                 acc[:, hr, :], mcast[:, 9, hr, :])

            for hh in range(2):
                hr = slice(hh * (HS // 2), (hh + 1) * (HS // 2))
                nc.sync.dma_start(out=out[:, hh * HS : (hh + 1) * HS, :],
                                  in_=out32[:, hr])

    nc.compile()
    return nc


def _get_program():
    global _PROGRAM
    if _PROGRAM is None:
        _PROGRAM = _build_program()
    return _PROGRAM


def _shard_inputs(x, comp_w, comp_b, enc_w, enc_b):
    comp_wT = np.ascontiguousarray(comp_w[:, :, 0, 0].T.astype(np.float16))
    enc_wT = np.ascontiguousarray(
        np.transpose(enc_w.reshape(KC, CC, 9), (1, 2, 0)).astype(np.float16))
    cb = np.ascontiguousarray(comp_b.astype(np.float32).reshape(CC, 1))
    eb = np.ascontiguousarray(enc_b.astype(np.float32).reshape(KC, 1))
    in_maps = []
    for core in range(NCORES):
        b, h = divmod(core, 2)
        xs = np.zeros((C, HS + 2, W), dtype=np.float32)
        lo = h * HS - 1
        s0, s1 = max(0, lo), min(H, lo + HS + 2)
        xs[:, s0 - lo : s1 - lo, :] = x[b, :, s0:s1, :]
        in_maps.append({
            "xs": np.ascontiguousarray(xs),
            "comp_wT": comp_wT,
            "comp_b": cb,
            "enc_wT": enc_wT,
            "enc_b": eb,
        })
    return in_maps


def _run(inputs, trace=False):
    from concourse.bass_utils import run_bass_kernel_spmd

    nc = _get_program()
    in_maps = _shard_inputs(**inputs)
    res = run_bass_kernel_spmd(nc, in_maps, list(range(NCORES)), trace=trace)
    out = np.empty((B, C, 2 * H, 2 * W), dtype=np.float32)
    for core in range(NCORES):
        b, h = divmod(core, 2)
        out[b, :, h * 2 * HS : (h + 1) * 2 * HS, :] = res.results[core]["out"]
    return out, res.exec_time_ns


def kernel(x, comp_w, comp_b, enc_w, enc_b):
    out, _ = _run(dict(x=np.asarray(x), comp_w=np.asarray(comp_w),
                       comp_b=np.asarray(comp_b), enc_w=np.asarray(enc_w),
                       enc_b=np.asarray(enc_b)))
    return out
